# revision 38
# baseline (speedup 1.0000x reference)
"""Distributed multi-head attention kernel for one TRN2 chip (8 NeuronCores).

Problem: B=4, S=2048, D=1024, H=16, Dh=64 fp32 attention
    q,k,v = x@W* + b*  (per head)  ->  softmax(q k^T / sqrt(Dh)) v  -> @Wo + bo

Sharding (per the hint): data-parallel over B (4) x tensor-parallel over
head-halves (2) = 8 cores.  Core c = 2*b + hg handles batch b and heads
[8*hg, 8*hg+8) i.e. d_model slice [512*hg, 512*hg+512).  Each core produces
a partial output [2048, 1024] (its 8 heads' contribution through Wo); the
host sums the two partials per batch and adds bo (the unshard step).

Per-core pipeline:
  - features-on-partitions: Q^T/K^T [dc, S] from the QKV matmuls; scores^T
    tiles land [k_seq, q_seq] with k on partitions.
  - softmax: exp() unnormalized on the Act engine; row-sums come from
    ones-columns appended to V (free: matmul cost is free-dim-bound).
  - normalization (1/rowsum) via reciprocal_approx_fast (custom DVE op,
    ~5x faster than InstReciprocal).  NOTE: custom-DVE ops silently
    no-op on partition-offset APs — always issue them on [0:128].
  - schedule: Q/K m0-n0 projections + first scores go FIRST so exp starts
    ~15us in; V projections fill block 0; remaining Q/K m-groups spread
    over blocks 1-12 (each m-group just before its consuming pair) and
    the output projection over blocks 13-15, sized so each step's PE
    work stays near the 2.2us of exp it feeds.  Final 2MB of output DMA
    fans out over 3 queues (Act's queue is idle by then).

Compute dtypes: bf16 matmul operands, fp32 PSUM accumulate, bf16 output
partials (host sums in fp32).  Measured rel-err ~2.7e-3 (gate 2e-2).
fp8 DoubleRow/SwInterleave AV was tried and REVERTED: on real TRN2 those
matmuls run at ~2x the duration of bf16 (no throughput win, cost model
disagrees with HW) and plain-DR needs no layout change but wins nothing.
"""

import sys

sys.path.insert(0, "/opt/trn_rl_repo")

import numpy as np
import ml_dtypes

from contextlib import ExitStack

import concourse.bass as bass
import concourse.tile as tile
from concourse import bacc, mybir
from concourse.bass_utils import run_bass_kernel_spmd
from concourse.tile import add_dep_helper

BF16 = mybir.dt.bfloat16
F32 = mybir.dt.float32
FP8 = mybir.dt.float8e4
AF = mybir.ActivationFunctionType
DR = mybir.MatmulPerfMode.DoubleRow


def _install_ntff_hook():
    """Provide antenv.axon_hooks (missing in this image) so that
    run_bass_kernel_spmd(trace=True) can capture NTFF profiles via the
    axon PJRT .so's C ABI."""
    import types, ctypes, contextlib

    if "antenv.axon_hooks" in sys.modules:
        return
    so_path = "/opt/axon/libaxon_pjrt.so"
    mod = types.ModuleType("antenv.axon_hooks")
    _state = {"hook": None}

    def set_axon_ntff_profile_hook(h):
        _state["hook"] = h

    def get_axon_ntff_profile_hook():
        return _state["hook"]

    mod.set_axon_ntff_profile_hook = set_axon_ntff_profile_hook
    mod.get_axon_ntff_profile_hook = get_axon_ntff_profile_hook
    sys.modules["antenv.axon_hooks"] = mod
    import antenv

    antenv.axon_hooks = mod

    try:
        lib = ctypes.CDLL(so_path)
    except OSError:
        return
    if not hasattr(lib, "axon_start_nrt_profile"):
        return
    lib.axon_start_nrt_profile.argtypes = [
        ctypes.POINTER(ctypes.c_int64),
        ctypes.c_size_t,
    ]
    lib.axon_start_nrt_profile.restype = ctypes.c_int64
    lib.axon_stop_nrt_profile.argtypes = [ctypes.c_char_p]
    lib.axon_stop_nrt_profile.restype = ctypes.c_int64

    @contextlib.contextmanager
    def _hook(output_dir, device_ids):
        import jax

        jax.devices()
        if device_ids:
            ids = (ctypes.c_int64 * len(device_ids))(*device_ids)
            rc = lib.axon_start_nrt_profile(ids, len(device_ids))
        else:
            rc = lib.axon_start_nrt_profile(None, 0)
        if rc != 0:
            raise RuntimeError(f"axon_start_nrt_profile rc={rc}")
        try:
            yield
        finally:
            n = lib.axon_stop_nrt_profile(str(output_dir).encode())
            print(f"profile: {n} file(s) written to {output_dir}",
                  file=sys.stderr)

    set_axon_ntff_profile_hook(_hook)


_install_ntff_hook()

D = 1024          # d_model
DC = 512          # per-core d slice (8 heads)
H_CORE = 8        # heads per core
DH = 64           # head dim
NPAIRS = 4        # head pairs per core


def build_graph(S=2048):
    """Build the per-core Bass graph (same graph on all 8 cores)."""
    nc = bacc.Bacc(
        "TRN2",
        target_bir_lowering=False,
        debug=False,
        enable_asserts=False,
        num_devices=8,
    )

    ST = S // 128       # 128-seq tiles (16)
    T2 = ST // 2        # 256-seq k-pair tiles (8)
    QT_ = S // 512      # 512-seq q blocks (4)

    xT = nc.dram_tensor("xT", [D, S], BF16, kind="ExternalInput").ap()
    # wqm/wkm are m-major and pre-transposed to the SBUF tile layout
    # [m-group, p, kt*128+c] so one contiguous DMA brings the whole
    # m-slice needed by a projection group.
    wqm = nc.dram_tensor("wqm", [4, 128, D], BF16, kind="ExternalInput").ap()
    wkm = nc.dram_tensor("wkm", [4, 128, D], BF16, kind="ExternalInput").ap()
    wv = nc.dram_tensor("wv", [D, DC], BF16, kind="ExternalInput").ap()
    wo = nc.dram_tensor("wo", [DC, D], BF16, kind="ExternalInput").ap()
    bq = nc.dram_tensor("bq", [DC, 1], F32, kind="ExternalInput").ap()
    bk = nc.dram_tensor("bk", [DC, 1], F32, kind="ExternalInput").ap()
    bvb = nc.dram_tensor("bvb", [128, DC], BF16, kind="ExternalInput").ap()
    # bf16 output: halves the 8MB/core output DMA; the host sums the two
    # core partials in fp32.  Costs ~2e-3 extra rel-err, well under gate.
    out = nc.dram_tensor("out", [S, D], BF16, kind="ExternalOutput").ap()

    with tile.TileContext(nc) as tc, ExitStack() as ctx:
        # ---- persistent pools --------------------------------------------
        qt_pool = ctx.enter_context(tc.tile_pool(name="qt", bufs=4))
        kt_pool = ctx.enter_context(tc.tile_pool(name="kt", bufs=4))
        vaug_pool = ctx.enter_context(tc.tile_pool(name="vaug", bufs=ST))
        ctx_pool = ctx.enter_context(tc.tile_pool(name="ctxT", bufs=4))
        const_pool = ctx.enter_context(tc.tile_pool(name="consts", bufs=1))
        wo_pool = ctx.enter_context(tc.tile_pool(name="wo", bufs=4))
        xt_pool = ctx.enter_context(tc.tile_pool(name="xt", bufs=8))
        wqk_pool = ctx.enter_context(tc.tile_pool(name="wqk", bufs=8))
        wv_pool = ctx.enter_context(tc.tile_pool(name="wv", bufs=8))

        qt_tiles = [qt_pool.tile([128, S], BF16, tag="qt", name=f"qt{i}")
                    for i in range(4)]
        kt_tiles = [kt_pool.tile([128, S], BF16, tag="kt", name=f"ktt{i}")
                    for i in range(4)]
        # vaug[st]: [128 kpos, 8 heads x (64 V | 64 ones)] bf16
        vaug_tiles = [vaug_pool.tile([128, H_CORE * 128], BF16, tag="vaug",
                                     name=f"vaug{i}") for i in range(ST)]
        ctx_tiles = [ctx_pool.tile([128, S], BF16, tag="ctxT", name=f"ctxT{i}")
                     for i in range(4)]
        xt_tiles = [xt_pool.tile([128, S], BF16, tag="xt", name=f"xtt{i}")
                    for i in range(8)]
        wqm_tiles = [wqk_pool.tile([128, D], BF16, tag="wqk", name=f"wqm{m}")
                     for m in range(4)]
        wkm_tiles = [wqk_pool.tile([128, D], BF16, tag="wqk", name=f"wkm{m}")
                     for m in range(4)]
        wv_tiles = [wv_pool.tile([128, DC], BF16, tag="wv", name=f"wvt{i}")
                    for i in range(8)]
        wo_tiles = [wo_pool.tile([128, D], BF16, tag="wo", name=f"wot{i}")
                    for i in range(4)]

        bvb_sb = const_pool.tile([128, DC], BF16, tag="bvb")
        bq_sb = const_pool.tile([128, DC // 128], F32, tag="bq")
        bk_sb = const_pool.tile([128, DC // 128], F32, tag="bk")
        wu_sb = const_pool.tile([128, 512], BF16, tag="wu")

        # ---- DMA plan: 2 queues (sync + gpsimd), critical prefix first ---
        # prefix: wkm[0] + wqm[0] + biases, THEN xt cols 0:512 in kt order
        # -> the K m0-n0 projection matmuls start as soon as wkm[0] + xt[0]
        # land, consuming xt tiles as they stream in.
        # The SCALAR queue must stay clean: DGE configs (~600ns each,
        # FIFO-serialized against transfer completions) on the Act
        # sequencer delay the first exp to ~35us, which stalls psS slot
        # recycling and leaves the PE idle ~6us.
        half = 512
        qs_ = [nc.sync, nc.gpsimd]
        di = 0

        def dma(dst, srcap):
            nonlocal di
            qs_[di % 2].dma_start(dst, srcap)
            di += 1

        dma(wkm_tiles[0][:], wkm[0])
        dma(wqm_tiles[0][:], wqm[0])
        dma(bk_sb[:], bk.rearrange("(m p) o -> p (m o)", p=128))
        dma(bq_sb[:], bq.rearrange("(m p) o -> p (m o)", p=128))
        for kt_ in range(8):
            dma(xt_tiles[kt_][:, 0:half], xT[kt_ * 128:(kt_ + 1) * 128, 0:half])
        # second wave: V weights (V projections start ~9us in)
        for kt_ in range(8):
            dma(wv_tiles[kt_][:], wv[kt_ * 128:(kt_ + 1) * 128, :])
        dma(bvb_sb[:], bvb[:])
        for kt_ in range(8):
            dma(xt_tiles[kt_][:, half:2 * half],
                xT[kt_ * 128:(kt_ + 1) * 128, half:2 * half])
        # third wave: remaining W m-groups + xt quarters + wo
        for m in range(1, 4):
            dma(wkm_tiles[m][:], wkm[m])
            dma(wqm_tiles[m][:], wqm[m])
        for kt_ in range(8):
            dma(xt_tiles[kt_][:, 2 * half:3 * half],
                xT[kt_ * 128:(kt_ + 1) * 128, 2 * half:3 * half])
        for dt_ in range(4):
            dma(wo_tiles[dt_][:], wo[dt_ * 128:(dt_ + 1) * 128, :])
        for kt_ in range(8):
            dma(xt_tiles[kt_][:, 3 * half:4 * half],
                xT[kt_ * 128:(kt_ + 1) * 128, 3 * half:4 * half])

        # wu memset on DVE so the PE warmup starts ~8us in.  The vaug
        # memsets are NOT done here: 16 of them head-of-queue on the DVE
        # block the K00/Q00 evacuations that gate the first scores (and on
        # GpSimd they delay that queue's DMA configs).  Each is emitted
        # just before its v_group instead, pacing them through block 0.
        nc.vector.memset(wu_sb[:], 0.0)

        # ---- PSUM pools ---------------------------------------------------
        # psS(2x2 banks) scores | psC(2x1) AV accum | psB(2x1) projections,
        # later swapped for psO(2x1) + osb staging.
        psS_cm = tc.tile_pool(name="psS", bufs=2, space="PSUM")
        psS = psS_cm.__enter__()
        psC_cm = tc.tile_pool(name="psC", bufs=2, space="PSUM")
        psC = psC_cm.__enter__()
        psB_cm = tc.tile_pool(name="psB", bufs=2, space="PSUM")
        psB = psB_cm.__enter__()

        exp_cm = tc.tile_pool(name="exp", bufs=6)
        exp_pool = exp_cm.__enter__()
        rec_cm = tc.tile_pool(name="rec", bufs=4)
        rec_pool = rec_cm.__enter__()

        state = {"psO": None, "osb": None}

        # ---- building blocks ---------------------------------------------
        # mm(): plain PE matmul.  (A variant gating post-score matmuls on
        # the score pair via add_dep_helper was tried and REVERTED: the
        # 64<->128 switch tax is paid per score pair even when pairs run
        # back-to-back, and the ordering deps blocked the scheduler from
        # filling Act-paced stalls -> +35us.)
        def mm(*args, **kw):
            return nc.tensor.matmul(*args, **kw)

        def proj_group(wm_tiles, dst, b_sb, m, n):
            p = psB.tile([128, 512], F32, tag="psB", name=f"psb{m}_{n}")
            for kt_ in range(8):
                mm(
                    p[:],
                    wm_tiles[m][:, kt_ * 128:(kt_ + 1) * 128],
                    xt_tiles[kt_][:, n * 512:(n + 1) * 512],
                    start=(kt_ == 0), stop=(kt_ == 7),
                )
            nc.vector.tensor_scalar(
                dst[m][:, n * 512:(n + 1) * 512], p[:],
                b_sb[:, m:m + 1], None, op0=mybir.AluOpType.add,
            )

        def v_group(st):
            # ones columns of vaug are static: memset to 1.0 just before
            # the V columns are filled (paced, not head-of-queue).
            nc.vector.memset(vaug_tiles[st][:], 1.0)
            pv = psB.tile([128, DC], F32, tag="psB", name=f"psv{st}")
            for kt_ in range(8):
                mm(
                    pv[:],
                    xt_tiles[kt_][:, st * 128:(st + 1) * 128],
                    wv_tiles[kt_][:],
                    start=(kt_ == 0), stop=(kt_ == 7),
                )
            # bias folded into the evacuation add (bvb is bv broadcast
            # across partitions, sent by the host)
            vt = vaug_tiles[st]
            nc.vector.tensor_add(
                vt[:].rearrange("p (h w) -> p h w", h=H_CORE)[:, :, 0:64],
                pv[:].rearrange("p (h w) -> p h w", h=H_CORE),
                bvb_sb[:].rearrange("p (h w) -> p h w", h=H_CORE),
            )

        def open_psO():
            psB_cm.__exit__(None, None, None)
            psO_cm = tc.tile_pool(name="psO", bufs=2, space="PSUM")
            state["psO"] = (psO_cm, psO_cm.__enter__())
            osb_cm = tc.tile_pool(name="osb", bufs=3)
            state["osb"] = (osb_cm, osb_cm.__enter__())

        def out_group(st, use_scalar=False, copy_eng=None):
            ss = slice(st * 128, (st + 1) * 128)
            psO = state["psO"][1]
            osb_pool = state["osb"][1]
            o_sb = osb_pool.tile([128, D], BF16, tag="osb", name=f"osb{st}")
            # at the tail the Act engine is idle, so its DMA queue helps
            # drain the last 2MB of output faster
            engs = ([nc.sync, nc.gpsimd, nc.scalar, nc.sync] if use_scalar
                    else [nc.sync, nc.gpsimd])
            ce_ = copy_eng or nc.vector

            def ce(dst, src):
                if ce_ is nc.scalar:
                    ce_.copy(dst, src)
                else:
                    ce_.tensor_copy(dst, src)
            for nh in range(2):
                po = psO.tile([128, 512], F32, tag="psO", name=f"po{st}_{nh}")
                for dt_ in range(4):
                    mm(
                        po[:],
                        ctx_tiles[dt_][:, ss],
                        wo_tiles[dt_][:, nh * 512:(nh + 1) * 512],
                        start=(dt_ == 0), stop=(dt_ == 3),
                    )
                if use_scalar:
                    # per-quarter copy+DMA so the final output transfers
                    # start as early as possible
                    hw = 256
                    for j in range(2):
                        c0 = nh * 512 + j * hw
                        ce(o_sb[:, c0:c0 + hw], po[:, j * hw:(j + 1) * hw])
                        engs[2 * nh + j].dma_start(
                            out[ss, c0:c0 + hw], o_sb[:, c0:c0 + hw])
                else:
                    ce(o_sb[:, nh * 512:(nh + 1) * 512], po[:])
                    engs[nh].dma_start(out[ss, nh * 512:(nh + 1) * 512],
                                       o_sb[:, nh * 512:(nh + 1) * 512])

        # ---- filler schedule ---------------------------------------------
        # blk = 4*p_ + q (16 blocks of 8 t2-steps).  Values: list of
        # callables per (blk, t2).
        inject = {}

        def add(blk, t2, fn):
            inject.setdefault((blk, t2), []).append(fn)

        # block 0: V projections, pair k at step k+1 (the lag-2 AV emission
        # means pair k isn't consumed until step 2k+2, so the first two
        # steps' scores/exp flow before the PE can stall on the wv DMAs).
        for k in range(7):
            if k > 0:
                add(0, min(k + 1, 7), lambda st=2 * k: v_group(st))
            add(0, min(k + 1, 7), lambda st=2 * k + 1: v_group(st))
        add(0, 7, lambda: v_group(14))
        add(0, 7, lambda: v_group(15))
        add(0, 1, lambda: proj_group(wkm_tiles, kt_tiles, bk_sb, 0, 1))
        add(0, 3, lambda: proj_group(wkm_tiles, kt_tiles, bk_sb, 0, 2))
        add(0, 5, lambda: proj_group(wkm_tiles, kt_tiles, bk_sb, 0, 3))
        add(0, 7, lambda: proj_group(wqm_tiles, qt_tiles, bq_sb, 0, 1))
        add(1, 0, lambda: proj_group(wqm_tiles, qt_tiles, bq_sb, 0, 2))
        add(1, 2, lambda: proj_group(wqm_tiles, qt_tiles, bq_sb, 0, 3))
        # remaining m1..3 groups, spread so each m-group lands just before
        # its consuming pair: m1 over blks 1-3, m2 over 4-7, m3 over 8-11.
        # This keeps early blocks from crowding out the scores->exp stream.
        rest = []
        for m in range(1, 4):
            for n in range(4):
                rest.append((wkm_tiles, kt_tiles, bk_sb, m, n))
            for n in range(4):
                rest.append((wqm_tiles, qt_tiles, bq_sb, m, n))
        slots = ([(1, t2) for t2 in (2, 4, 6)] +
                 [(2, t2) for t2 in (0, 2, 4, 6)] + [(3, 0)] +
                 [(blk, t2) for blk in range(4, 8) for t2 in (2, 6)] +
                 [(8, 2), (8, 6), (9, 2), (9, 6)] +
                 [(10, 2), (11, 2), (12, 2), (12, 4)])
        for (blk, t2), (wt, dst, bs, m, n) in zip(slots, rest):
            add(blk, t2,
                lambda wt=wt, dst=dst, bs=bs, m=m, n=n:
                    proj_group(wt, dst, bs, m, n))
        add(13, 0, open_psO)
        # output projection: q-block j outputs mostly during block 13+j,
        # at ODD groups so they do not wait on the block's deferred pair-3
        # normalize (DVE) which lands during group 0.  One already-ready
        # out_group from the previous q-block fills group 0 of blocks
        # 14/15; out 10/11 are held back for the final tail, where they
        # fill the PE while the DVE runs the last normalize chunks.
        for st, slot in [(0, (13, 1)), (1, (13, 3)), (2, (13, 5)),
                         (3, (13, 7)), (4, (14, 1)), (5, (14, 3)),
                         (6, (14, 5)), (7, (14, 7)), (8, (15, 1)),
                         (9, (15, 3))]:
            add(slot[0], slot[1], lambda st=st: out_group(st))

        # ---- main loop ----------------------------------------------------
        # PE warmup: ~24 dummy matmuls with no DMA dependencies keep the
        # Tensor engine pipeline dense through the DMA-paced startup window
        # (first real inputs land ~10us in on HW).  Without this the first
        # real matmuls run with multi-us gaps that keep resetting the PE
        # p-state ramp (0.65/1.2 GHz instead of 2.4 -> 427-585ns slices).
        # Sized so the warmup ends ~21us in (HW), by which point the DMA
        # prefix + wv + xt-n1 waves have landed and the real stream never
        # stalls (a stalled PE resets the p-state ramp: 427ns matmuls).
        # warmup PSUM comes from psC (first real psC write is the first AV
        # at ~28us): taking a psB slot would make v_group(0) wait on the
        # K00 evacuation and leave a ~2us PE bubble at ~25us.
        wu_ps = psC.tile([128, 512], F32, tag="psC", name="wups")
        for i in range(48):
            nc.tensor.matmul(
                wu_ps[:], wu_sb[:, 0:128], wu_sb[:],
                start=(i == 0), stop=(i == 47),
            )

        # pre-work: K/Q m0-n0 so the first exp fires as early as possible.
        # v_group(0) after them fills the ~1.4us PE bubble while the Q00
        # PSUM->SBUF evacuation (DVE) that gates the first scores runs.
        proj_group(wkm_tiles, kt_tiles, bk_sb, 0, 0)
        proj_group(wqm_tiles, qt_tiles, bq_sb, 0, 0)
        v_group(0)

        # Cross-block pipelining: each block's last two AV pairs and its
        # normalize are deferred into the NEXT block's first step (after
        # its first scores), so the in-order PE never waits for the last
        # exps of a block.  They must be emitted BEFORE that step's
        # injections: out_groups depend on the normalize's ctx writes
        # through the in-order DVE queue.
        pending = []
        for p_ in range(NPAIRS):
            for q in range(QT_):
                blk = 4 * p_ + q
                qs = slice(q * 512, (q + 1) * 512)
                pc0 = psC.tile([128, 512], F32, tag="psC", name=f"pc0_{p_}_{q}")
                pc1 = psC.tile([128, 512], F32, tag="psC", name=f"pc1_{p_}_{q}")
                # AV matmuls are emitted TWO kt-steps behind the scores
                # (software pipelining): the PE runs in program order, so
                # by the time it reaches AV(kt) the Act engine finished
                # exp(kt) ~400ns earlier and the PE never stalls on it.
                def emit_av(kt_, e, pc0=pc0, pc1=pc1, p_=p_):
                    mm(
                        pc0[:],
                        vaug_tiles[kt_][:, (2 * p_) * 128:(2 * p_ + 1) * 128],
                        e[:, 0:512],
                        start=(kt_ == 0), stop=(kt_ == ST - 1),
                    )
                    mm(
                        pc1[:],
                        vaug_tiles[kt_][:, (2 * p_ + 1) * 128:(2 * p_ + 2) * 128],
                        e[:, 512:1024],
                        start=(kt_ == 0), stop=(kt_ == ST - 1),
                    )

                def normalize(pc0=pc0, pc1=pc1, p_=p_, q=q, qs=qs):
                    cps = []
                    for h, pc in ((0, pc0), (1, pc1)):
                        cp = rec_pool.tile([128, 1024], F32, tag="cp",
                                           name=f"cp{p_}_{q}_{h}")
                        nc.vector.tensor_copy(cp[64:128, 0:512], pc[0:64, :])
                        nc.vector.tensor_copy(cp[64:128, 512:1024],
                                              pc[64:128, :])
                        cps.append(cp)
                    for h, cp in ((0, cps[0]), (1, cps[1])):
                        rec = rec_pool.tile([128, 512], F32, tag="rec",
                                            name=f"rec{p_}_{q}_{h}")
                        # custom-DVE ops require full-partition APs; rows
                        # 0:64 of cp are stale garbage; rec[0:64] unused.
                        nc.vector.reciprocal_approx_fast(rec[:, :],
                                                         cp[:, 512:1024])
                        nc.vector.tensor_mul(
                            ctx_tiles[p_][h * 64:(h + 1) * 64, qs],
                            cp[64:128, 0:512], rec[64:128, :],
                        )

                # scores for TWO kt-steps back-to-back (batch-2): halves the
                # 64<->128 PE tile-config switches, so only one AV per group
                # pays the ~100ns post-switch weight-load stall instead of
                # one per kt-step.
                def emit_scores(kt_):
                    ks = slice(kt_ * 128, (kt_ + 1) * 128)
                    ps = psS.tile([128, 1024], F32, tag="psS",
                                  name=f"ps{p_}_{q}_{kt_}")
                    nc.tensor.matmul(
                        ps[:, 0:512], kt_tiles[p_][0:64, ks],
                        qt_tiles[p_][0:64, qs],
                        start=True, stop=True, tile_position=(0, 0),
                    )
                    s2 = nc.tensor.matmul(
                        ps[:, 512:1024], kt_tiles[p_][64:128, ks],
                        qt_tiles[p_][64:128, qs],
                        start=True, stop=True, tile_position=(64, 0),
                    )
                    e = exp_pool.tile([128, 1024], BF16, tag="exp",
                                      name=f"e{p_}_{q}_{kt_}")
                    nc.scalar.activation(e[:], ps[:], AF.Exp, scale=0.125)
                    es_hold[kt_] = e
                    return s2

                es_hold = {}
                for g in range(ST // 2):
                    emit_scores(2 * g)
                    if g == 0:
                        # between the first two score pairs the PE waits
                        # ~1us for the psS slots (recycled by the previous
                        # block's last exps): fill with an out_group whose
                        # inputs are already complete.
                        for fn in inject.get((blk, "mid"), []):
                            fn()
                    emit_scores(2 * g + 1)
                    if g == 0:
                        for fn in pending:
                            fn()
                        pending = []
                    for fn in inject.get((blk, g), []):
                        fn()
                    if g >= 1:
                        emit_av(2 * g - 2, es_hold.pop(2 * g - 2))
                        emit_av(2 * g - 1, es_hold.pop(2 * g - 1))

                last = (p_ == NPAIRS - 1 and q == QT_ - 1)
                if not last:
                    e14, e15 = es_hold.pop(ST - 2), es_hold.pop(ST - 1)
                    pending = [
                        lambda f=emit_av, e=e14: f(ST - 2, e),
                        lambda f=emit_av, e=e15: f(ST - 1, e),
                        normalize,
                    ]
                else:
                    for kt_ in (ST - 2, ST - 1):
                        emit_av(kt_, es_hold.pop(kt_))
                    # q-block-2 outputs only need the pair-3 q2 normalize
                    # (done at this block's group 0): they keep the PE busy
                    # while the DVE runs the final normalize chunks.  Tail
                    # PSUM->SBUF copies go to the Act engine (free after
                    # the last exp; GpSimd cannot read PSUM) so the DVE is
                    # clear for the normalize chain.
                    out_group(10, use_scalar=True, copy_eng=nc.scalar)
                    out_group(11, use_scalar=True, copy_eng=nc.scalar)
                    # last block: normalize in 256-col chunks and start the
                    # q3 output projections as soon as their columns are
                    # ready, pipelining the tail DVE chain with the PE.
                    # (128-col chunks were tried: more small-op overhead,
                    # +1us.)
                    for ci in range(2):
                        cs = slice(q * 512 + ci * 256, q * 512 + ci * 256 + 256)
                        pcs_ = slice(ci * 256, ci * 256 + 256)
                        for h, pc in ((0, pc0), (1, pc1)):
                            cp = rec_pool.tile([128, 512], F32, tag="cp",
                                               name=f"cpL{ci}_{h}")
                            nc.vector.tensor_copy(cp[64:128, 0:256],
                                                  pc[0:64, pcs_])
                            nc.vector.tensor_copy(cp[64:128, 256:512],
                                                  pc[64:128, pcs_])
                            rec = rec_pool.tile([128, 256], F32, tag="rec",
                                                name=f"recL{ci}_{h}")
                            nc.vector.reciprocal_approx_fast(rec[:, :],
                                                             cp[:, 256:512])
                            nc.vector.tensor_mul(
                                ctx_tiles[p_][h * 64:(h + 1) * 64, cs],
                                cp[64:128, 0:256], rec[64:128, :],
                            )
                        out_group(12 + 2 * ci, use_scalar=True,
                                  copy_eng=nc.scalar)
                        out_group(13 + 2 * ci, use_scalar=True,
                                  copy_eng=nc.scalar)

        state["osb"][0].__exit__(None, None, None)
        state["psO"][0].__exit__(None, None, None)
        rec_cm.__exit__(None, None, None)
        exp_cm.__exit__(None, None, None)
        psC_cm.__exit__(None, None, None)
        psS_cm.__exit__(None, None, None)

    nc.finalize()
    return nc


_CACHED = {}


def _get_graph(S):
    if S not in _CACHED:
        _CACHED[S] = build_graph(S)
    return _CACHED[S]


def make_in_maps(x, Wq, bq, Wk, bk, Wv, bv, Wo, bo):
    bf = ml_dtypes.bfloat16
    in_maps = []
    for c in range(8):
        b, hg = c // 2, c % 2
        sl = slice(512 * hg, 512 * (hg + 1))
        # [m, p, kt*128+c]: W[d=kt*128+p, m*128+c] -> wq_m[m, p, kt, c]
        wq_m = np.ascontiguousarray(
            Wq[:, sl].reshape(8, 128, 4, 128).transpose(2, 1, 0, 3)
            .reshape(4, 128, D)).astype(bf)
        wk_m = np.ascontiguousarray(
            Wk[:, sl].reshape(8, 128, 4, 128).transpose(2, 1, 0, 3)
            .reshape(4, 128, D)).astype(bf)
        in_maps.append({
            "xT": np.ascontiguousarray(x[b].T).astype(bf),
            "wqm": wq_m,
            "wkm": wk_m,
            "wv": np.ascontiguousarray(Wv[:, sl]).astype(bf),
            "wo": np.ascontiguousarray(Wo[sl, :]).astype(bf),
            "bq": np.ascontiguousarray(bq[sl]).reshape(512, 1).astype(np.float32),
            "bk": np.ascontiguousarray(bk[sl]).reshape(512, 1).astype(np.float32),
            "bvb": np.ascontiguousarray(
                np.broadcast_to(bv[sl].reshape(1, 512), (128, 512))).astype(bf),
        })
    return in_maps


def kernel(x, Wq, bq, Wk, bk, Wv, bv, Wo, bo, _trace=False, _tmpdir=None):
    x = np.asarray(x, dtype=np.float32)
    S = x.shape[1]
    nc = _get_graph(S)
    in_maps = make_in_maps(x, np.asarray(Wq), np.asarray(bq), np.asarray(Wk),
                           np.asarray(bk), np.asarray(Wv), np.asarray(bv),
                           np.asarray(Wo), np.asarray(bo))
    res = run_bass_kernel_spmd(
        nc, in_maps, core_ids=list(range(8)), trace=_trace, tmpdir=_tmpdir,
    )
    bo32 = np.asarray(bo, dtype=np.float32)
    outs = [np.asarray(r["out"], dtype=np.float32) for r in res.results]
    full = np.stack([outs[2 * b] + outs[2 * b + 1] + bo32 for b in range(4)])
    kernel.last_results = res
    return full

